# revision 17
# baseline (speedup 1.0000x reference)
"""DebertaV2 MoE layer (top-2 of 8 experts, BERT-style FFN + per-expert
LayerNorm) on 8 Trainium2 NeuronCores, expert-parallel: one expert per core.

Host side: router (softmax/top-2/renorm) decides the token->expert dispatch
and combine weights; tokens are gathered per expert, padded to a fixed
capacity C, and shipped to the expert's core together with that expert's
weights. Device side (SPMD, one Bass/Tile program on all 8 cores): the
expert FFN (x@w1 -> gelu -> @w2 -> +residual -> LayerNorm -> *combine
weight) plus a data-parallel slice of the router matmul (so router logits
are device-computed too). Host scatters the per-expert outputs back.

Matmul layout: mm1 computes hT = w1.T @ xT (intermediate stays
token-minor), mm2 computes y = hT.T @ w2 which lands token-major so the
LayerNorm reduction runs along the free axis. FFN matmuls use float32r
(full-rate fp32 streaming mode); router matmul uses plain float32.
"""

import math
import os

import numpy as np

B, S, H, I, E, KTOP = 2, 2048, 1024, 4096, 8, 2
T = B * S
P = 128
LN_EPS = 1e-7
NCORES = 8
R = T // NCORES  # tokens per core for the data-parallel router slice
KT1 = H // P  # 8   k-tiles for mm1 (contraction over H)
M1 = I // P  # 32  output row-tiles of mm1 / k-tiles of mm2
NH = 2  # H split into two 512-wide psum banks for mm2

_NC_CACHE = {}
LAST_PERF = None  # BassKernelResults of the most recent run (for test harness)


def _split_subs(chunk):
    """Split a token chunk into matmul free-dim substreams <=512, each >=256
    (float32r runs 4x slower below a 256-wide moving operand)."""
    assert chunk % P == 0
    subs = []
    rem = chunk
    while rem > 512:
        take = 512 if rem - 512 == 0 or rem - 512 >= 256 else 384
        subs.append(take)
        rem -= take
    subs.append(rem)
    assert sum(subs) == chunk and all(s >= 256 or s == chunk for s in subs)
    return subs


def _plan_chunks(C):
    """Token chunks: each <=1024 (psum banks: chunk/128 + 2 <= 8 wants
    <=768; 1024 still works since mm2 runs nh-major with chunk/128 banks),
    multiples of 128."""
    n = math.ceil(C / 768)
    base = math.ceil(C / n / P) * P
    chunks = []
    rem = C
    while rem > 0:
        take = min(base, rem)
        chunks.append(take)
        rem -= take
    assert sum(chunks) == C
    return chunks


def _build_nc(C, chunks, have_g, have_b):
    import concourse.bass as bass
    import concourse.mybir as mybir
    import concourse.tile as tile
    from concourse import bacc
    from contextlib import ExitStack

    f32 = mybir.dt.float32
    f32r = mybir.dt.float32r
    i32 = mybir.dt.int32
    AF = mybir.ActivationFunctionType
    OP = mybir.AluOpType

    nc = bacc.Bacc(None, target_bir_lowering=False)

    xt = nc.dram_tensor("xt", [H, C], f32r, kind="ExternalInput")
    xr = nc.dram_tensor("xr", [C, H], f32, kind="ExternalInput")
    w1cb = nc.dram_tensor("w1cb", [M1, P, KT1, P], f32r, kind="ExternalInput")
    b1c = nc.dram_tensor("b1c", [P, M1], f32, kind="ExternalInput")
    w2 = nc.dram_tensor("w2", [I, H], f32r, kind="ExternalInput")
    lng = nc.dram_tensor("lng", [1, H], f32, kind="ExternalInput")
    lnb = nc.dram_tensor("lnb", [1, H], f32, kind="ExternalInput")
    wcol = nc.dram_tensor("wcol", [P, C // P], f32, kind="ExternalInput")
    rxt = nc.dram_tensor("rxt", [H, R], f32, kind="ExternalInput")
    rwt = nc.dram_tensor("rwt", [P, KT1, E], f32, kind="ExternalInput")
    y = nc.dram_tensor("y", [C, H], f32, kind="ExternalOutput")
    rlog = nc.dram_tensor("rlog", [E, R], f32, kind="ExternalOutput")

    with tile.TileContext(nc) as tc, ExitStack() as ctx:
        const = ctx.enter_context(tc.tile_pool(name="const", bufs=1))
        rpool = ctx.enter_context(tc.tile_pool(name="rpool", bufs=3))
        ps = ctx.enter_context(tc.tile_pool(name="ps", bufs=8, space="PSUM"))
        xt_pool = ctx.enter_context(tc.tile_pool(name="xt_pool", bufs=12))
        w1_pool = ctx.enter_context(tc.tile_pool(name="w1_pool", bufs=5))
        ht_pool = ctx.enter_context(tc.tile_pool(name="ht_pool", bufs=M1))
        w2_pool = ctx.enter_context(tc.tile_pool(name="w2_pool", bufs=6))
        xr_pool = ctx.enter_context(tc.tile_pool(name="xr_pool", bufs=3))
        y_pool = ctx.enter_context(tc.tile_pool(name="y_pool", bufs=6))
        st_pool = ctx.enter_context(tc.tile_pool(name="st_pool", bufs=4))

        # b1 is needed by the very first gelu eviction; the LN constants are
        # not needed until much later, so they are loaded mid-kernel below.
        b1s = const.tile([P, M1], f32)
        nc.sync.dma_start(out=b1s, in_=b1c[:])
        g_bc = const.tile([P, H], f32) if have_g else None
        b_bc = const.tile([P, H], f32) if have_b else None
        wc = const.tile([P, C // P], f32)
        rw_sb = const.tile([P, KT1, E], f32)
        consts_loaded = False

        # --- router FIRST: its tiny DMAs land fast, so the PE warms up on
        # router matmuls (~7us of fp32 work) while the big mm1 working set
        # streams in. rlog[e, r] = sum_h rw[h, e] * x[h, r]
        nc.sync.dma_start(out=rw_sb, in_=rwt[:])
        rps = ps.tile([E, R], f32, tag="ps")
        for kt in range(KT1):
            rx_t = rpool.tile([P, R], f32, tag="rx")
            nc.sync.dma_start(out=rx_t, in_=rxt[kt * P : (kt + 1) * P, :])
            nc.tensor.matmul(
                rps,
                lhsT=rw_sb[:, kt, :],
                rhs=rx_t,
                start=(kt == 0),
                stop=(kt == KT1 - 1),
            )
        rlog_sb = rpool.tile([E, R], f32)
        nc.vector.tensor_copy(out=rlog_sb, in_=rps)
        nc.sync.dma_start(out=rlog[:, :], in_=rlog_sb)

        xt_v = xt[:].rearrange("(kt p) c -> p kt c", p=P)

        c0 = 0
        for ci, chunk in enumerate(chunks):
            subs = _split_subs(chunk)
            n_ct = chunk // P

            # per-kt loads so mm1 m=0 can start as soon as k-tile 0 lands
            xt_kts = []
            for kt in range(KT1):
                xt_k = xt_pool.tile([P, chunk], f32r, tag="xt", name="xt_k")
                nc.scalar.dma_start(out=xt_k, in_=xt_v[:, kt, c0 : c0 + chunk])
                xt_kts.append(xt_k)

            # --- mm1: hT[m] = gelu(w1[:, m].T @ xT + b1[m]) ---
            ht_tiles = []
            for m in range(M1):
                w1_t = w1_pool.tile([P, KT1, P], f32r, tag="w1")
                nc.sync.dma_start(out=w1_t, in_=w1cb[m, :, :, :])
                psums = []
                s0 = 0
                for sub in subs:
                    p1 = ps.tile([P, sub], f32, tag="ps")
                    psums.append((p1, s0, sub))
                    s0 += sub
                for kt in range(KT1):
                    lw = w1_t[:, kt, :]
                    for p1, s0, sub in psums:
                        nc.tensor.matmul(
                            p1,
                            lhsT=lw,
                            rhs=xt_kts[kt][:, s0 : s0 + sub],
                            start=(kt == 0),
                            stop=(kt == KT1 - 1),
                        )
                ht_t = ht_pool.tile([P, chunk], f32r, tag="ht")
                for p1, s0, sub in psums:
                    nc.scalar.activation(
                        out=ht_t[:, s0 : s0 + sub],
                        in_=p1,
                        func=AF.Gelu,
                        bias=b1s[:, m : m + 1],
                        scale=1.0,
                    )
                ht_tiles.append(ht_t)
                if ci == 0 and m == 0 and not consts_loaded:
                    # LN constants: needed first ~80us in; emitting here keeps
                    # their DMAs off the critical head
                    consts_loaded = True
                    if have_g:
                        nc.sync.dma_start(out=g_bc, in_=lng[:].to_broadcast([P, H]))
                    if have_b:
                        nc.sync.dma_start(out=b_bc, in_=lnb[:].to_broadcast([P, H]))
                    nc.sync.dma_start(out=wc, in_=wcol[:])

            # --- residual inputs + y tiles ---
            xr_ts = []
            y_ts = []
            for ct in range(n_ct):
                xr_t = xr_pool.tile([P, H], f32, tag="xr")
                nc.scalar.dma_start(
                    out=xr_t, in_=xr[c0 + ct * P : c0 + (ct + 1) * P, :]
                )
                xr_ts.append(xr_t)
                y_t = y_pool.tile([P, H], f32, tag="y", name="y_t")
                y_ts.append(y_t)

            # --- mm2: y[ct, nh] = sum_kt2 hT[kt2][:, ct].T @ w2[kt2, nh] ---
            for nh in range(NH):
                ps2 = {}
                for kt2 in range(M1):
                    w2_t = w2_pool.tile([P, 512], f32r, tag="w2")
                    nc.sync.dma_start(
                        out=w2_t,
                        in_=w2[kt2 * P : (kt2 + 1) * P, nh * 512 : (nh + 1) * 512],
                    )
                    w2r = w2_t[:]
                    for ct in range(n_ct):
                        if kt2 == 0:
                            ps2[ct] = ps.tile([P, 512], f32, tag="ps", name="ps2")
                        nc.tensor.matmul(
                            ps2[ct],
                            lhsT=ht_tiles[kt2][:, ct * P : (ct + 1) * P],
                            rhs=w2r,
                            start=(kt2 == 0),
                            stop=(kt2 == M1 - 1),
                        )
                for ct in range(n_ct):
                    nc.vector.tensor_add(
                        out=y_ts[ct][:, nh * 512 : (nh + 1) * 512],
                        in0=ps2[ct],
                        in1=xr_ts[ct][:, nh * 512 : (nh + 1) * 512],
                    )

            # --- LayerNorm + combine-weight scale ---
            # stats for all c-tiles of the chunk, then ONE batched
            # Newton-rsqrt over [P, n_ct] instead of n_ct serial chains
            mvall = st_pool.tile([P, n_ct, 2], f32, tag="mv", name="mvall")
            for ct in range(n_ct):
                y_t = y_ts[ct]
                stats = st_pool.tile([P, 2, 6], f32, tag="st")
                nc.vector.bn_stats(out=stats[:, 0, :], in_=y_t[:, 0:512])
                nc.vector.bn_stats(out=stats[:, 1, :], in_=y_t[:, 512:1024])
                nc.vector.bn_aggr(out=mvall[:, ct, :], in_=stats)
            # rstd = 1/sqrt(var + eps) via bit-trick seed + 3 Newton steps
            # (keeps Sqrt off the ACT engine -> no act-table switches)
            v = st_pool.tile([P, n_ct], f32, tag="v", name="v")
            nc.vector.tensor_scalar_add(out=v, in0=mvall[:, :, 1], scalar1=LN_EPS)
            r = st_pool.tile([P, n_ct], f32, tag="r", name="r")
            nc.vector.tensor_scalar(
                out=r.bitcast(i32),
                in0=v.bitcast(i32),
                scalar1=1,
                scalar2=None,
                op0=OP.logical_shift_right,
            )
            nc.vector.tensor_scalar(
                out=r.bitcast(i32),
                in0=r.bitcast(i32),
                scalar1=-1,
                scalar2=0x5F3759DF,
                op0=OP.mult,
                op1=OP.add,
            )
            t1 = st_pool.tile([P, n_ct], f32, tag="t1", name="t1")
            for _ in range(3):
                nc.vector.tensor_mul(out=t1, in0=r, in1=r)
                nc.vector.tensor_mul(out=t1, in0=t1, in1=v)
                nc.vector.tensor_scalar(
                    out=t1,
                    in0=t1,
                    scalar1=-0.5,
                    scalar2=1.5,
                    op0=OP.mult,
                    op1=OP.add,
                )
                nc.vector.tensor_mul(out=r, in0=r, in1=t1)
            # fold the combine weights into rstd: A = rstd * w
            nc.vector.tensor_mul(
                out=r, in0=r, in1=wc[:, c0 // P : c0 // P + n_ct]
            )
            for ct in range(n_ct):
                y_t = y_ts[ct]
                ctg = c0 // P + ct
                # y = (y - mean) * A    [A = w * rstd]
                nc.vector.tensor_scalar(
                    out=y_t,
                    in0=y_t,
                    scalar1=mvall[:, ct, 0:1],
                    scalar2=r[:, ct : ct + 1],
                    op0=OP.subtract,
                    op1=OP.mult,
                )
                if have_g:
                    # y *= gamma
                    nc.vector.tensor_mul(out=y_t, in0=y_t, in1=g_bc)
                if have_b:
                    # y += w * beta
                    nc.vector.scalar_tensor_tensor(
                        out=y_t,
                        in0=b_bc,
                        scalar=wc[:, ctg : ctg + 1],
                        in1=y_t,
                        op0=OP.mult,
                        op1=OP.add,
                    )
                nc.scalar.dma_start(
                    out=y[c0 + ct * P : c0 + (ct + 1) * P, :], in_=y_t
                )

            c0 += chunk

    nc.compile()
    return nc


def _get_nc(C, chunks, have_g, have_b):
    key = (C, tuple(chunks), have_g, have_b)
    if key not in _NC_CACHE:
        _NC_CACHE[key] = _build_nc(C, chunks, have_g, have_b)
    return _NC_CACHE[key]


def _ensure_axon_hooks():
    """This image's ``antenv`` package lacks the ``axon_hooks`` submodule
    that ``bass_utils`` imports when trace=True under axon. Reconstruct it:
    a get/set pair plus the ctypes NTFF-profile hook into libaxon_pjrt.so
    (same ABI trn_agent_boot.trn_boot uses)."""
    try:
        import antenv.axon_hooks  # noqa: F401

        return
    except ImportError:
        pass
    import contextlib
    import ctypes
    import sys
    import types

    import antenv

    mod = types.ModuleType("antenv.axon_hooks")
    mod._hook = None

    def set_axon_ntff_profile_hook(h):
        mod._hook = h

    def get_axon_ntff_profile_hook():
        return mod._hook

    mod.set_axon_ntff_profile_hook = set_axon_ntff_profile_hook
    mod.get_axon_ntff_profile_hook = get_axon_ntff_profile_hook
    sys.modules["antenv.axon_hooks"] = mod
    antenv.axon_hooks = mod

    so_path = "/opt/axon/libaxon_pjrt.so"
    if os.path.exists(so_path):
        try:
            lib = ctypes.CDLL(so_path)
            if hasattr(lib, "axon_start_nrt_profile"):
                lib.axon_start_nrt_profile.argtypes = [
                    ctypes.POINTER(ctypes.c_int64),
                    ctypes.c_size_t,
                ]
                lib.axon_start_nrt_profile.restype = ctypes.c_int64
                lib.axon_stop_nrt_profile.argtypes = [ctypes.c_char_p]
                lib.axon_stop_nrt_profile.restype = ctypes.c_int64

                @contextlib.contextmanager
                def _hook(output_dir, device_ids):
                    import jax

                    jax.devices()
                    if device_ids:
                        ids = (ctypes.c_int64 * len(device_ids))(*device_ids)
                        rc = lib.axon_start_nrt_profile(ids, len(device_ids))
                    else:
                        rc = lib.axon_start_nrt_profile(None, 0)
                    if rc != 0:
                        raise RuntimeError(f"axon_start_nrt_profile rc={rc}")
                    try:
                        yield
                    finally:
                        n = lib.axon_stop_nrt_profile(str(output_dir).encode())
                        print(
                            f"ntff profile: {n} file(s) written to {output_dir}",
                            file=sys.stderr,
                        )

                mod._hook = _hook
        except OSError:
            pass


def kernel(**inputs):
    global LAST_PERF
    _ensure_axon_hooks()
    from concourse.bass_utils import run_bass_kernel_spmd

    x = np.ascontiguousarray(np.asarray(inputs["x"], dtype=np.float32))
    router_w = np.ascontiguousarray(np.asarray(inputs["router_w"], dtype=np.float32))
    w1 = np.asarray(inputs["w1"], dtype=np.float32)
    b1 = np.asarray(inputs["b1"], dtype=np.float32)
    w2 = np.asarray(inputs["w2"], dtype=np.float32)
    b2 = np.asarray(inputs["b2"], dtype=np.float32)
    ln_g = np.asarray(inputs["ln_g"], dtype=np.float32)
    ln_b = np.asarray(inputs["ln_b"], dtype=np.float32)

    t = x.reshape(T, H)

    # --- host router: dispatch decision + combine weights ---
    logits = (t @ router_w).astype(np.float64)
    pe = np.exp(logits - logits.max(-1, keepdims=True))
    probs = pe / pe.sum(-1, keepdims=True)
    sel = np.argsort(-probs, axis=1, kind="stable")[:, :KTOP]
    rw_k = np.take_along_axis(probs, sel, 1)
    rw_k = rw_k / rw_k.sum(-1, keepdims=True)
    w_full = np.zeros((T, E), dtype=np.float64)
    np.put_along_axis(w_full, sel, rw_k, axis=1)
    w_full = w_full.astype(np.float32)

    idx = [np.nonzero((sel == e).any(axis=1))[0] for e in range(E)]
    maxc = max(len(ix) for ix in idx)
    C = max(512, math.ceil(maxc / P) * P)
    chunks = _plan_chunks(C)

    have_g = bool(np.any(ln_g != 1.0))
    have_b = bool(np.any(ln_b != 0.0))
    nc = _get_nc(C, chunks, have_g, have_b)

    tT = np.ascontiguousarray(t.T)  # [H, T]

    in_maps = []
    for e in range(E):
        ix = idx[e]
        n = len(ix)
        xt_e = np.zeros((H, C), dtype=np.float32)
        xt_e[:, :n] = tT[:, ix]
        xr_e = np.zeros((C, H), dtype=np.float32)
        xr_e[:n] = t[ix] + b2[e][None, :]
        w1cb_e = np.ascontiguousarray(
            w1[e].reshape(KT1, P, M1, P).transpose(2, 1, 0, 3)
        )
        b1c_e = np.ascontiguousarray(b1[e].reshape(M1, P).T)
        w2_e = np.ascontiguousarray(w2[e])
        wslot = np.zeros(C, dtype=np.float32)
        wslot[:n] = w_full[ix, e]
        wcol_e = np.ascontiguousarray(wslot.reshape(C // P, P).T)
        rxt_e = np.ascontiguousarray(tT[:, e * R : (e + 1) * R])
        rwt_e = np.ascontiguousarray(router_w.reshape(KT1, P, E).transpose(1, 0, 2))
        in_maps.append(
            {
                "xt": xt_e,
                "xr": xr_e,
                "w1cb": w1cb_e,
                "b1c": b1c_e,
                "w2": w2_e,
                "lng": np.ascontiguousarray(ln_g[e].reshape(1, H)),
                "lnb": np.ascontiguousarray(ln_b[e].reshape(1, H)),
                "wcol": wcol_e,
                "rxt": rxt_e,
                "rwt": rwt_e,
            }
        )

    res = run_bass_kernel_spmd(nc, in_maps, core_ids=list(range(NCORES)))
    LAST_PERF = res

    out = np.zeros((T, H), dtype=np.float32)
    for e in range(E):
        ix = idx[e]
        out[ix] += res.results[e]["y"][: len(ix)]
    logits_out = np.concatenate(
        [res.results[c]["rlog"].T for c in range(NCORES)], axis=0
    )
    return out.reshape(B, S, H), np.ascontiguousarray(logits_out).reshape(B, S, E)


# revision 20
# speedup vs baseline: 1.0457x; 1.0457x over previous
"""DebertaV2 MoE layer (top-2 of 8 experts, BERT-style FFN + per-expert
LayerNorm) on 8 Trainium2 NeuronCores, expert-parallel: one expert per core.

Host side: router (softmax/top-2/renorm) decides the token->expert dispatch
and combine weights; tokens are gathered per expert, padded to a fixed
capacity C, and shipped to the expert's core together with that expert's
weights. Device side (SPMD, one Bass/Tile program on all 8 cores): the
expert FFN (x@w1 -> gelu -> @w2 -> +residual -> LayerNorm -> *combine
weight) plus a data-parallel slice of the router matmul (so router logits
are device-computed too). Host scatters the per-expert outputs back.

Matmul layout: mm1 computes hT = w1.T @ xT (intermediate stays
token-minor), mm2 computes y = hT.T @ w2 which lands token-major so the
LayerNorm reduction runs along the free axis. FFN matmuls use float32r
(full-rate fp32 streaming mode); router matmul uses plain float32.
"""

import math
import os

import numpy as np

B, S, H, I, E, KTOP = 2, 2048, 1024, 4096, 8, 2
T = B * S
P = 128
LN_EPS = 1e-7
NCORES = 8
R = T // NCORES  # tokens per core for the data-parallel router slice
KT1 = H // P  # 8   k-tiles for mm1 (contraction over H)
M1 = I // P  # 32  output row-tiles of mm1 / k-tiles of mm2
NH = 2  # H split into two 512-wide psum banks for mm2

_NC_CACHE = {}
LAST_PERF = None  # BassKernelResults of the most recent run (for test harness)


def _split_subs(chunk):
    """Split a token chunk into matmul free-dim substreams <=512, each >=256
    (float32r runs 4x slower below a 256-wide moving operand)."""
    assert chunk % P == 0
    subs = []
    rem = chunk
    while rem > 512:
        take = 512 if rem - 512 == 0 or rem - 512 >= 256 else 384
        subs.append(take)
        rem -= take
    subs.append(rem)
    assert sum(subs) == chunk and all(s >= 256 or s == chunk for s in subs)
    return subs


def _plan_chunks(C):
    """Token chunks, multiples of 128, each <=768 (mm2 holds chunk/128
    psum banks, + 2 for the next chunk's mm1 <= 8). Greedy 768s: a big
    leading chunk maximizes 512-wide mm1 substreams (best LDW hiding)
    and leaves a small final chunk (short LayerNorm tail)."""
    n = math.ceil(C / 640)
    base = math.ceil(C / n / P) * P
    chunks = []
    rem = C
    while rem > 0:
        take = min(base, rem)
        chunks.append(take)
        rem -= take
    assert sum(chunks) == C and all(c <= 640 for c in chunks)
    return chunks


def _build_nc(C, chunks, have_g, have_b):
    import concourse.bass as bass
    import concourse.mybir as mybir
    import concourse.tile as tile
    from concourse import bacc
    from contextlib import ExitStack

    f32 = mybir.dt.float32
    f32r = mybir.dt.float32r
    i32 = mybir.dt.int32
    AF = mybir.ActivationFunctionType
    OP = mybir.AluOpType

    nc = bacc.Bacc(None, target_bir_lowering=False)

    xt = nc.dram_tensor("xt", [H, C], f32r, kind="ExternalInput")
    xr = nc.dram_tensor("xr", [C, H], f32, kind="ExternalInput")
    w1cb = nc.dram_tensor("w1cb", [M1, P, KT1, P], f32r, kind="ExternalInput")
    b1c = nc.dram_tensor("b1c", [P, M1], f32, kind="ExternalInput")
    w2 = nc.dram_tensor("w2", [I, H], f32r, kind="ExternalInput")
    lng = nc.dram_tensor("lng", [1, H], f32, kind="ExternalInput")
    lnb = nc.dram_tensor("lnb", [1, H], f32, kind="ExternalInput")
    wcol = nc.dram_tensor("wcol", [P, C // P], f32, kind="ExternalInput")
    rxt = nc.dram_tensor("rxt", [H, R], f32, kind="ExternalInput")
    rwt = nc.dram_tensor("rwt", [P, KT1, E], f32, kind="ExternalInput")
    y = nc.dram_tensor("y", [C, H], f32, kind="ExternalOutput")
    rlog = nc.dram_tensor("rlog", [E, R], f32, kind="ExternalOutput")

    with tile.TileContext(nc) as tc, ExitStack() as ctx:
        const = ctx.enter_context(tc.tile_pool(name="const", bufs=1))
        rpool = ctx.enter_context(tc.tile_pool(name="rpool", bufs=3))
        ps = ctx.enter_context(tc.tile_pool(name="ps", bufs=8, space="PSUM"))
        xt_pool = ctx.enter_context(tc.tile_pool(name="xt_pool", bufs=10))
        w1_pool = ctx.enter_context(tc.tile_pool(name="w1_pool", bufs=5))
        ht_pool = ctx.enter_context(tc.tile_pool(name="ht_pool", bufs=M1))
        w2_pool = ctx.enter_context(tc.tile_pool(name="w2_pool", bufs=6))
        xr_pool = ctx.enter_context(tc.tile_pool(name="xr_pool", bufs=3))
        y_pool = ctx.enter_context(tc.tile_pool(name="y_pool", bufs=6))
        st_pool = ctx.enter_context(tc.tile_pool(name="st_pool", bufs=4))

        # b1 is needed by the very first gelu eviction; the LN constants are
        # not needed until much later, so they are loaded mid-kernel below.
        b1s = const.tile([P, M1], f32)
        nc.sync.dma_start(out=b1s, in_=b1c[:])
        g_bc = const.tile([P, H], f32) if have_g else None
        b_bc = const.tile([P, H], f32) if have_b else None
        wc = const.tile([P, C // P], f32)
        rw_sb = const.tile([P, KT1, E], f32)
        consts_loaded = False

        # --- router FIRST: its tiny DMAs land fast, so the PE warms up on
        # router matmuls (~7us of fp32 work) while the big mm1 working set
        # streams in. rlog[e, r] = sum_h rw[h, e] * x[h, r]
        nc.sync.dma_start(out=rw_sb, in_=rwt[:])
        rps = ps.tile([E, R], f32, tag="ps")
        for kt in range(KT1):
            rx_t = rpool.tile([P, R], f32, tag="rx")
            nc.sync.dma_start(out=rx_t, in_=rxt[kt * P : (kt + 1) * P, :])
            nc.tensor.matmul(
                rps,
                lhsT=rw_sb[:, kt, :],
                rhs=rx_t,
                start=(kt == 0),
                stop=(kt == KT1 - 1),
            )
        rlog_sb = rpool.tile([E, R], f32)
        nc.vector.tensor_copy(out=rlog_sb, in_=rps)
        nc.sync.dma_start(out=rlog[:, :], in_=rlog_sb)

        xt_v = xt[:].rearrange("(kt p) c -> p kt c", p=P)

        c0 = 0
        for ci, chunk in enumerate(chunks):
            subs = _split_subs(chunk)
            n_ct = chunk // P

            # per-kt loads so mm1 m=0 can start as soon as k-tile 0 lands
            xt_kts = []
            for kt in range(KT1):
                xt_k = xt_pool.tile([P, chunk], f32r, tag="xt", name="xt_k")
                nc.sync.dma_start(out=xt_k, in_=xt_v[:, kt, c0 : c0 + chunk])
                xt_kts.append(xt_k)

            # --- mm1: hT[m] = gelu(w1[:, m].T @ xT + b1[m]) ---
            ht_tiles = []
            for m in range(M1):
                w1_t = w1_pool.tile([P, KT1, P], f32r, tag="w1")
                nc.sync.dma_start(out=w1_t, in_=w1cb[m, :, :, :])
                psums = []
                s0 = 0
                for sub in subs:
                    p1 = ps.tile([P, sub], f32, tag="ps")
                    psums.append((p1, s0, sub))
                    s0 += sub
                for kt in range(KT1):
                    lw = w1_t[:, kt, :]
                    for p1, s0, sub in psums:
                        nc.tensor.matmul(
                            p1,
                            lhsT=lw,
                            rhs=xt_kts[kt][:, s0 : s0 + sub],
                            start=(kt == 0),
                            stop=(kt == KT1 - 1),
                        )
                ht_t = ht_pool.tile([P, chunk], f32r, tag="ht")
                for p1, s0, sub in psums:
                    nc.scalar.activation(
                        out=ht_t[:, s0 : s0 + sub],
                        in_=p1,
                        func=AF.Gelu,
                        bias=b1s[:, m : m + 1],
                        scale=1.0,
                    )
                ht_tiles.append(ht_t)
                if ci == 0 and m == 0 and not consts_loaded:
                    # LN constants: needed first ~80us in; emitting here keeps
                    # their DMAs off the critical head
                    consts_loaded = True
                    if have_g:
                        nc.sync.dma_start(out=g_bc, in_=lng[:].to_broadcast([P, H]))
                    if have_b:
                        nc.sync.dma_start(out=b_bc, in_=lnb[:].to_broadcast([P, H]))
                    nc.sync.dma_start(out=wc, in_=wcol[:])

            # --- residual inputs + y tiles ---
            xr_ts = []
            y_ts = []
            for ct in range(n_ct):
                xr_t = xr_pool.tile([P, H], f32, tag="xr")
                nc.sync.dma_start(
                    out=xr_t, in_=xr[c0 + ct * P : c0 + (ct + 1) * P, :]
                )
                xr_ts.append(xr_t)
                y_t = y_pool.tile([P, H], f32, tag="y", name="y_t")
                y_ts.append(y_t)

            # --- mm2: y[ct, nh] = sum_kt2 hT[kt2][:, ct].T @ w2[kt2, nh] ---
            for nh in range(NH):
                ps2 = {}
                for kt2 in range(M1):
                    w2_t = w2_pool.tile([P, 512], f32r, tag="w2")
                    nc.sync.dma_start(
                        out=w2_t,
                        in_=w2[kt2 * P : (kt2 + 1) * P, nh * 512 : (nh + 1) * 512],
                    )
                    w2r = w2_t[:]
                    for ct in range(n_ct):
                        if kt2 == 0:
                            ps2[ct] = ps.tile([P, 512], f32, tag="ps", name="ps2")
                        nc.tensor.matmul(
                            ps2[ct],
                            lhsT=ht_tiles[kt2][:, ct * P : (ct + 1) * P],
                            rhs=w2r,
                            start=(kt2 == 0),
                            stop=(kt2 == M1 - 1),
                        )
                for ct in range(n_ct):
                    nc.vector.tensor_add(
                        out=y_ts[ct][:, nh * 512 : (nh + 1) * 512],
                        in0=ps2[ct],
                        in1=xr_ts[ct][:, nh * 512 : (nh + 1) * 512],
                    )

            # --- LayerNorm + combine-weight scale ---
            # stats for all c-tiles of the chunk, then ONE batched
            # Newton-rsqrt over [P, n_ct] instead of n_ct serial chains
            mvall = st_pool.tile([P, n_ct, 2], f32, tag="mv", name="mvall")
            for ct in range(n_ct):
                y_t = y_ts[ct]
                stats = st_pool.tile([P, 2, 6], f32, tag="st")
                nc.vector.bn_stats(out=stats[:, 0, :], in_=y_t[:, 0:512])
                nc.vector.bn_stats(out=stats[:, 1, :], in_=y_t[:, 512:1024])
                nc.vector.bn_aggr(out=mvall[:, ct, :], in_=stats)
            # rstd = 1/sqrt(var + eps) via bit-trick seed + 3 Newton steps
            # (keeps Sqrt off the ACT engine -> no act-table switches)
            v = st_pool.tile([P, n_ct], f32, tag="v", name="v")
            nc.vector.tensor_scalar_add(out=v, in0=mvall[:, :, 1], scalar1=LN_EPS)
            r = st_pool.tile([P, n_ct], f32, tag="r", name="r")
            nc.vector.tensor_scalar(
                out=r.bitcast(i32),
                in0=v.bitcast(i32),
                scalar1=1,
                scalar2=None,
                op0=OP.logical_shift_right,
            )
            nc.vector.tensor_scalar(
                out=r.bitcast(i32),
                in0=r.bitcast(i32),
                scalar1=-1,
                scalar2=0x5F3759DF,
                op0=OP.mult,
                op1=OP.add,
            )
            t1 = st_pool.tile([P, n_ct], f32, tag="t1", name="t1")
            for _ in range(3):
                nc.vector.tensor_mul(out=t1, in0=r, in1=r)
                nc.vector.tensor_mul(out=t1, in0=t1, in1=v)
                nc.vector.tensor_scalar(
                    out=t1,
                    in0=t1,
                    scalar1=-0.5,
                    scalar2=1.5,
                    op0=OP.mult,
                    op1=OP.add,
                )
                nc.vector.tensor_mul(out=r, in0=r, in1=t1)
            # fold the combine weights into rstd: A = rstd * w
            nc.vector.tensor_mul(
                out=r, in0=r, in1=wc[:, c0 // P : c0 // P + n_ct]
            )
            for ct in range(n_ct):
                y_t = y_ts[ct]
                ctg = c0 // P + ct
                # y = (y - mean) * A    [A = w * rstd]
                nc.vector.tensor_scalar(
                    out=y_t,
                    in0=y_t,
                    scalar1=mvall[:, ct, 0:1],
                    scalar2=r[:, ct : ct + 1],
                    op0=OP.subtract,
                    op1=OP.mult,
                )
                if have_g:
                    # y *= gamma
                    nc.vector.tensor_mul(out=y_t, in0=y_t, in1=g_bc)
                if have_b:
                    # y += w * beta
                    nc.vector.scalar_tensor_tensor(
                        out=y_t,
                        in0=b_bc,
                        scalar=wc[:, ctg : ctg + 1],
                        in1=y_t,
                        op0=OP.mult,
                        op1=OP.add,
                    )
                nc.sync.dma_start(
                    out=y[c0 + ct * P : c0 + (ct + 1) * P, :], in_=y_t
                )

            c0 += chunk

    nc.compile()
    return nc


def _get_nc(C, chunks, have_g, have_b):
    key = (C, tuple(chunks), have_g, have_b)
    if key not in _NC_CACHE:
        _NC_CACHE[key] = _build_nc(C, chunks, have_g, have_b)
    return _NC_CACHE[key]


def _ensure_axon_hooks():
    """This image's ``antenv`` package lacks the ``axon_hooks`` submodule
    that ``bass_utils`` imports when trace=True under axon. Reconstruct it:
    a get/set pair plus the ctypes NTFF-profile hook into libaxon_pjrt.so
    (same ABI trn_agent_boot.trn_boot uses)."""
    try:
        import antenv.axon_hooks  # noqa: F401

        return
    except ImportError:
        pass
    import contextlib
    import ctypes
    import sys
    import types

    import antenv

    mod = types.ModuleType("antenv.axon_hooks")
    mod._hook = None

    def set_axon_ntff_profile_hook(h):
        mod._hook = h

    def get_axon_ntff_profile_hook():
        return mod._hook

    mod.set_axon_ntff_profile_hook = set_axon_ntff_profile_hook
    mod.get_axon_ntff_profile_hook = get_axon_ntff_profile_hook
    sys.modules["antenv.axon_hooks"] = mod
    antenv.axon_hooks = mod

    so_path = "/opt/axon/libaxon_pjrt.so"
    if os.path.exists(so_path):
        try:
            lib = ctypes.CDLL(so_path)
            if hasattr(lib, "axon_start_nrt_profile"):
                lib.axon_start_nrt_profile.argtypes = [
                    ctypes.POINTER(ctypes.c_int64),
                    ctypes.c_size_t,
                ]
                lib.axon_start_nrt_profile.restype = ctypes.c_int64
                lib.axon_stop_nrt_profile.argtypes = [ctypes.c_char_p]
                lib.axon_stop_nrt_profile.restype = ctypes.c_int64

                @contextlib.contextmanager
                def _hook(output_dir, device_ids):
                    import jax

                    jax.devices()
                    if device_ids:
                        ids = (ctypes.c_int64 * len(device_ids))(*device_ids)
                        rc = lib.axon_start_nrt_profile(ids, len(device_ids))
                    else:
                        rc = lib.axon_start_nrt_profile(None, 0)
                    if rc != 0:
                        raise RuntimeError(f"axon_start_nrt_profile rc={rc}")
                    try:
                        yield
                    finally:
                        n = lib.axon_stop_nrt_profile(str(output_dir).encode())
                        print(
                            f"ntff profile: {n} file(s) written to {output_dir}",
                            file=sys.stderr,
                        )

                mod._hook = _hook
        except OSError:
            pass


def kernel(**inputs):
    global LAST_PERF
    _ensure_axon_hooks()
    from concourse.bass_utils import run_bass_kernel_spmd

    x = np.ascontiguousarray(np.asarray(inputs["x"], dtype=np.float32))
    router_w = np.ascontiguousarray(np.asarray(inputs["router_w"], dtype=np.float32))
    w1 = np.asarray(inputs["w1"], dtype=np.float32)
    b1 = np.asarray(inputs["b1"], dtype=np.float32)
    w2 = np.asarray(inputs["w2"], dtype=np.float32)
    b2 = np.asarray(inputs["b2"], dtype=np.float32)
    ln_g = np.asarray(inputs["ln_g"], dtype=np.float32)
    ln_b = np.asarray(inputs["ln_b"], dtype=np.float32)

    t = x.reshape(T, H)

    # --- host router: dispatch decision + combine weights ---
    logits = (t @ router_w).astype(np.float64)
    pe = np.exp(logits - logits.max(-1, keepdims=True))
    probs = pe / pe.sum(-1, keepdims=True)
    sel = np.argsort(-probs, axis=1, kind="stable")[:, :KTOP]
    rw_k = np.take_along_axis(probs, sel, 1)
    rw_k = rw_k / rw_k.sum(-1, keepdims=True)
    w_full = np.zeros((T, E), dtype=np.float64)
    np.put_along_axis(w_full, sel, rw_k, axis=1)
    w_full = w_full.astype(np.float32)

    idx = [np.nonzero((sel == e).any(axis=1))[0] for e in range(E)]
    maxc = max(len(ix) for ix in idx)
    C = max(512, math.ceil(maxc / P) * P)
    chunks = _plan_chunks(C)

    have_g = bool(np.any(ln_g != 1.0))
    have_b = bool(np.any(ln_b != 0.0))
    nc = _get_nc(C, chunks, have_g, have_b)

    tT = np.ascontiguousarray(t.T)  # [H, T]

    in_maps = []
    for e in range(E):
        ix = idx[e]
        n = len(ix)
        xt_e = np.zeros((H, C), dtype=np.float32)
        xt_e[:, :n] = tT[:, ix]
        xr_e = np.zeros((C, H), dtype=np.float32)
        xr_e[:n] = t[ix] + b2[e][None, :]
        w1cb_e = np.ascontiguousarray(
            w1[e].reshape(KT1, P, M1, P).transpose(2, 1, 0, 3)
        )
        b1c_e = np.ascontiguousarray(b1[e].reshape(M1, P).T)
        w2_e = np.ascontiguousarray(w2[e])
        wslot = np.zeros(C, dtype=np.float32)
        wslot[:n] = w_full[ix, e]
        wcol_e = np.ascontiguousarray(wslot.reshape(C // P, P).T)
        rxt_e = np.ascontiguousarray(tT[:, e * R : (e + 1) * R])
        rwt_e = np.ascontiguousarray(router_w.reshape(KT1, P, E).transpose(1, 0, 2))
        in_maps.append(
            {
                "xt": xt_e,
                "xr": xr_e,
                "w1cb": w1cb_e,
                "b1c": b1c_e,
                "w2": w2_e,
                "lng": np.ascontiguousarray(ln_g[e].reshape(1, H)),
                "lnb": np.ascontiguousarray(ln_b[e].reshape(1, H)),
                "wcol": wcol_e,
                "rxt": rxt_e,
                "rwt": rwt_e,
            }
        )

    res = run_bass_kernel_spmd(nc, in_maps, core_ids=list(range(NCORES)))
    LAST_PERF = res

    out = np.zeros((T, H), dtype=np.float32)
    for e in range(E):
        ix = idx[e]
        out[ix] += res.results[e]["y"][: len(ix)]
    logits_out = np.concatenate(
        [res.results[c]["rlog"].T for c in range(NCORES)], axis=0
    )
    return out.reshape(B, S, H), np.ascontiguousarray(logits_out).reshape(B, S, E)
